# revision 1
# baseline (speedup 1.0000x reference)
"""Causal MHA (B=1, T=4096, D=768, H=12) on 8 TRN2 NeuronCores.

Strategy
--------
- Sequence-parallel over T with row-interleaved q-assignment so every core
  runs the *same* program on identically-shaped causal work:
  core c owns q rows {16*(c+8*t)+u}, i.e. 16-row miniblocks strided by 8.
  Each core also owns the contiguous K/V chunk rows [512c, 512c+512).
- No collectives: each core redundantly projects the FULL K^T (SBUF
  resident) and V' (via local DRAM) from the full x — measured cheaper
  than the AllGather (whose entry barrier + RDH transfer cost ~165us) and
  immune to cross-core launch stagger. Attention uses the S^T = K.Q^T
  layout (keys on partitions, q on free axis) so the softmax denominator
  folds into the PV matmul via a ones-column on V'.
- Scores are ~N(0,1) (x ~ N(0,1), W ~ N(0,1)/sqrt(D)), so softmax skips
  the running-max entirely: exp(s/8) never overflows fp32.
- All matmuls run in float32r (full-rate fp32 PE mode, ~1.6e-4 rel err).
- Causal masking is multiplicative on P^T after exp, using per-core mask
  tiles streamed in as inputs (the only rank-dependent data besides the
  shards themselves).
"""
import sys

sys.path.insert(0, "/opt/trn_rl_repo")

import numpy as np

import concourse.bass as bass
import concourse.mybir as mybir
import concourse.tile as tile
from concourse.bass_utils import run_bass_kernel_spmd

P = 128
T, D, H, HD = 4096, 768, 12, 64
NC = 8
SQ = T // NC          # 512 q rows per core
SKV = T // NC         # 512 kv rows per core
DC = D // P           # 6 contraction chunks
NKB = T // P          # 32 key blocks of 128
VROW = H * (HD + 1)   # 780: V' row with ones col per head
KSZ = D * SKV         # K^T shard elems
VSZ = SKV * VROW      # V' shard elems
F32R = mybir.dt.float32r
F32 = mybir.dt.float32

# kb batches: (kbs, mlo, N, SLOT). Matmul PSUM outputs must not cross a
# 512-col bank boundary, so slots are strided by 512 when N=384.
_BATCHES = []
for _mlo, _G in ((0, (2, 2, 2, 2)), (1, (2, 2, 2, 2)), (2, (4, 4)), (3, (8,))):
    _kb = 8 * _mlo
    _n = 512 - 128 * _mlo
    _slot = 512 if _n > 256 else _n
    for _g in _G:
        _BATCHES.append((list(range(_kb, _kb + _g)), _mlo, _n, _slot))
        _kb += _g


def q_rows(c):
    t = np.arange(32)
    u = np.arange(16)
    return (16 * (c + 8 * t)[:, None] + u[None, :]).reshape(-1)


def make_masks(c):
    r = np.arange(8)[:, None, None]
    kap = np.arange(128)[None, :, None]
    j = np.arange(128)[None, None, :]
    valid = (128 * r + kap) <= (16 * c + 128 * (j // 16) + (j % 16))
    return valid.astype(np.float32)


def fix_excess_waits(nc):
    """walrus rejects >1 sync wait per instruction; hoist extras onto NoOps."""
    k = 0
    for f in nc.m.functions:
        for bb in f.blocks:
            insts = bb.instructions
            i = 0
            while i < len(insts):
                ins = insts[i]
                si = getattr(ins, "sync_info", None)
                if si is not None and len(si.on_wait) > 1:
                    for w in si.on_wait[:-1]:
                        nop = mybir.InstNoOp(name=f"W-hoist-{k}", ins=[], outs=[])
                        k += 1
                        nop.engine = ins.engine
                        nop.sync_info = mybir.SyncInfo(on_wait=[w], on_update=[])
                        insts.insert(i, nop)
                        i += 1
                    ins.sync_info = mybir.SyncInfo(
                        on_wait=[si.on_wait[-1]], on_update=list(si.on_update))
                i += 1
    return k


def build(fix_waits=True):
    nc = bass.Bass()
    xqt = nc.dram_tensor("xqt", [D, SQ], F32R, kind="ExternalInput")
    xt = nc.dram_tensor("xt", [D, T], F32R, kind="ExternalInput")
    wq = nc.dram_tensor("wq", [D, D], F32R, kind="ExternalInput")
    wk = nc.dram_tensor("wk", [D, D], F32R, kind="ExternalInput")
    wv = nc.dram_tensor("wv", [D, D], F32R, kind="ExternalInput")
    wo = nc.dram_tensor("wo", [D, D], F32R, kind="ExternalInput")
    bo = nc.dram_tensor("bo", [P, D], F32R, kind="ExternalInput")
    masks = nc.dram_tensor("masks", [8, P, P], F32R, kind="ExternalInput")
    out = nc.dram_tensor("out", [SQ, D], F32, kind="ExternalOutput")

    EXP = mybir.ActivationFunctionType.Exp

    with tile.TileContext(nc) as tc:
        with (
            tc.tile_pool(name="glob", bufs=1) as glob,
            tc.tile_pool(name="dram", bufs=1, space="DRAM") as dram,
            tc.tile_pool(name="kt", bufs=1) as ktp,
        ):
            # ---- tiles that live the whole kernel
            qt_z = glob.tile([P, H, SQ], F32R)       # zero-padded per-head Q^T
            masks_sb = glob.tile([P, 8, P], F32R)
            bo_bc = glob.tile([P, D], F32R)
            vfull = [dram.tile([VSZ], F32R, name=f"vfull{r}") for r in range(NC)]

            nc.sync.dma_start(masks_sb[:], masks.rearrange("r p j -> p r j"))
            nc.sync.dma_start(bo_bc[:], bo[:])
            nc.vector.memset(qt_z.bitcast(mybir.dt.uint32), 0)

            kt_c = [ktp.tile([P, DC, SKV], F32R, name=f"ktc{r}") for r in range(NC)]

            # ===== phase 1b: Q^T into zero-padded per-head slots
            with (
                tc.tile_pool(name="ph1b", bufs=1) as ph1b,
                tc.tile_pool(name="ps1b", bufs=2, space="PSUM") as ps1b,
            ):
                wq_sb = ph1b.tile([P, DC, D], F32R)
                xq_sb = ph1b.tile([P, DC, SQ], F32R)
                nc.sync.dma_start(wq_sb[:], wq.rearrange("(o p) d -> p o d", p=P))
                nc.sync.dma_start(xq_sb[:], xqt.rearrange("(o p) t -> p o t", p=P))
                for dc in range(DC):
                    pp = ps1b.tile([P, SQ], F32, tag="pp")
                    for ko in range(DC):
                        nc.tensor.matmul(
                            pp[:], wq_sb[:, ko, dc * P:(dc + 1) * P],
                            xq_sb[:, ko, :], start=(ko == 0), stop=(ko == DC - 1))
                    nc.vector.tensor_copy(qt_z[0:64, 2 * dc, :], pp[0:64, :])
                    nc.vector.tensor_copy(qt_z[64:128, 2 * dc + 1, :], pp[64:128, :])

            # ===== phase 1a: K^T and V' for the FULL sequence, per 512-chunk
            with (
                tc.tile_pool(name="ph1a", bufs=1) as ph1a,
                tc.tile_pool(name="xc", bufs=2) as xcp,
                tc.tile_pool(name="ps1", bufs=2, space="PSUM") as ps1,
            ):
                wk_sb = ph1a.tile([P, DC, D], F32R)
                wv_sb = ph1a.tile([P, DC, D], F32R)
                nc.sync.dma_start(wk_sb[:], wk.rearrange("(o p) d -> p o d", p=P))
                nc.sync.dma_start(wv_sb[:], wv.rearrange("(o p) d -> p o d", p=P))
                xtv = xt.rearrange("(o p) t -> p o t", p=P)

                for r in range(NC):
                    xtc = xcp.tile([P, DC, SKV], F32R, tag="xc")
                    nc.sync.dma_start(xtc[:], xtv[:, :, r * SKV:(r + 1) * SKV])
                    # K^T chunk -> straight into resident kt_c[r]
                    for dc in range(DC):
                        pp = ps1.tile([P, SKV], F32, tag="pp")
                        for ko in range(DC):
                            nc.tensor.matmul(
                                pp[:], wk_sb[:, ko, dc * P:(dc + 1) * P],
                                xtc[:, ko, :], start=(ko == 0), stop=(ko == DC - 1))
                        nc.vector.tensor_copy(kt_c[r][:, dc, :], pp[:])
                    # V' chunk -> local DRAM
                    v_st = ph1a.tile([P, SKV // P, VROW], F32R, tag="vst")
                    v4 = v_st.rearrange("p o (h c) -> p o h c", c=HD + 1)
                    nc.vector.memset(
                        v4[:, :, :, HD:HD + 1].bitcast(mybir.dt.uint32), 0x3F800000)
                    for tc4 in range(SKV // P):
                        for nh in range(2):
                            pp = ps1.tile([P, 384], F32, tag="ppv")
                            for ko in range(DC):
                                nc.tensor.matmul(
                                    pp[:], xtc[:, ko, tc4 * P:(tc4 + 1) * P],
                                    wv_sb[:, ko, nh * 384:(nh + 1) * 384],
                                    start=(ko == 0), stop=(ko == DC - 1))
                            nc.vector.tensor_copy(
                                v4[:, tc4, nh * 6:(nh + 1) * 6, 0:HD],
                                pp.rearrange("p (h c) -> p h c", c=HD))
                    nc.sync.dma_start(
                        vfull[r].rearrange("(o p c) -> p o c", p=P, c=VROW), v_st[:])

            # ================= phase 2 + 3 ==================================
            with tc.tile_pool(name="mid", bufs=1) as mid:
                ctxt = mid.tile([P, DC, SQ], F32R)   # ctx^T, d on partitions

                with (
                    tc.tile_pool(name="att", bufs=4) as att,
                    tc.tile_pool(name="vp", bufs=20) as vp,
                    tc.tile_pool(name="ps_s", bufs=3, space="PSUM") as ps_s,
                    tc.tile_pool(name="ps_c", bufs=2, space="PSUM") as ps_c,
                ):
                    # heads processed in pairs, batch-interleaved: the PE runs
                    # head h+1's S^T while ACT/DVE exp+mask head h's batch.
                    for h0 in range(0, H, 2):
                        scope = nc.named_scope(f"attn{h0}")
                        scope.__enter__()
                        pair = (h0, h0 + 1)
                        cps = {h: ps_c.tile([P, SQ], F32, tag="ctx",
                                            name=f"cps{h}") for h in pair}
                        vts = {h: {} for h in pair}
                        for kbs, mlo, N, SLOT in _BATCHES:
                            W = len(kbs) * SLOT
                            for h in pair:
                                hp = h // 2
                                sps = ps_s.tile([P, 1024], F32, tag="s")
                                for i, kb in enumerate(kbs):
                                    nc.tensor.matmul(
                                        sps[:, i * SLOT:i * SLOT + N],
                                        kt_c[kb // 4][:, hp,
                                                      (kb % 4) * P:(kb % 4 + 1) * P],
                                        qt_z[:, h, 128 * mlo:SQ],
                                        start=True, stop=True)
                                if SLOT != N:
                                    nc.vector.memset(
                                        sps[:, :W].rearrange("p (g s) -> p g s", s=SLOT)
                                        [:, :, N:SLOT].bitcast(mybir.dt.uint32), 0)
                                pt = att.tile([P, 1024], F32R, tag="pt")
                                nc.scalar.activation(
                                    pt[:, :W], sps[:, :W], EXP, scale=0.125)
                                ptv = pt[:, :W].rearrange("p (g n) -> p g n", n=SLOT)
                                r0 = kbs[0] - 8 * mlo
                                nc.vector.tensor_mul(
                                    ptv[:, :, 0:P], ptv[:, :, 0:P],
                                    masks_sb[:, r0:r0 + len(kbs), :])
                                for i, kb in enumerate(kbs):
                                    r = kb // 4
                                    if r not in vts[h]:
                                        vtr = vp.tile([P, 4, HD + 1], F32R, tag="v")
                                        nc.sync.dma_start(
                                            vtr[:],
                                            vfull[r]
                                            .rearrange("(o p c) -> p o c", p=P, c=VROW)
                                            [:, :, h * (HD + 1):(h + 1) * (HD + 1)])
                                        vts[h][r] = vtr
                                    nc.tensor.matmul(
                                        cps[h][0:HD + 1, 128 * mlo:SQ],
                                        vts[h][r][:, kb % 4, :],
                                        pt[:, i * SLOT:i * SLOT + N],
                                        start=(kb == 0), stop=(kb == NKB - 1),
                                        skip_group_check=True)
                        for h in pair:
                            hp, hr = h // 2, (h % 2) * 64
                            rec = att.tile([1, SQ], F32, tag="rec")
                            nc.vector.reciprocal(rec[:], cps[h][HD:HD + 1, :])
                            drec = dram.tile([1, SQ], F32, name=f"drec{h}")
                            nc.sync.dma_start(drec[:], rec[:])
                            bc = att.tile([64, SQ], F32, tag="bc")
                            nc.sync.dma_start(bc[:], drec.to_broadcast([64, SQ]))
                            nc.vector.tensor_mul(
                                ctxt[hr:hr + 64, hp, :], cps[h][0:64, :], bc[:])
                        scope.__exit__(None, None, None)

                # ---- output projection
                with (
                    tc.tile_pool(name="ph3", bufs=1) as ph3,
                    tc.tile_pool(name="ps3", bufs=2, space="PSUM") as ps3,
                ):
                    wo_sb = ph3.tile([P, DC, D], F32R)
                    nc.sync.dma_start(wo_sb[:], wo.rearrange("(o p) d -> p o d", p=P))
                    o_sb = ph3.tile([P, SQ // P, D], F32)
                    for tc4 in range(SQ // P):
                        for nh in range(2):
                            op = ps3.tile([P, 384], F32, tag="op")
                            for dc in range(DC):
                                nc.tensor.matmul(
                                    op[:], ctxt[:, dc, tc4 * P:(tc4 + 1) * P],
                                    wo_sb[:, dc, nh * 384:(nh + 1) * 384],
                                    start=(dc == 0), stop=(dc == DC - 1))
                            nc.vector.tensor_add(
                                o_sb[:, tc4, nh * 384:(nh + 1) * 384], op[:],
                                bo_bc[:, nh * 384:(nh + 1) * 384])
                    nc.sync.dma_start(
                        out.rearrange("(o p) d -> p o d", p=P), o_sb[:])

    if fix_waits:
        fix_excess_waits(nc)
    return nc


_NC_CACHE = None


def _get_nc():
    global _NC_CACHE
    if _NC_CACHE is None:
        _NC_CACHE = build()
    return _NC_CACHE


def _run(inputs, trace=False):
    x = np.asarray(inputs["x"], dtype=np.float32)
    Wq = np.asarray(inputs["Wq"], dtype=np.float32)
    Wk = np.asarray(inputs["Wk"], dtype=np.float32)
    Wv = np.asarray(inputs["Wv"], dtype=np.float32)
    Wo = np.asarray(inputs["Wo"], dtype=np.float32)
    bo_v = np.ascontiguousarray(
        np.broadcast_to(np.asarray(inputs["bo"], dtype=np.float32).reshape(1, D),
                        (P, D)))
    xf = x.reshape(T, D)

    nc_prog = _get_nc()
    xt_full = np.ascontiguousarray(xf.T)
    in_maps = []
    for c in range(NC):
        rows = q_rows(c)
        in_maps.append({
            "xqt": np.ascontiguousarray(xf[rows].T),
            "xt": xt_full,
            "wq": Wq, "wk": Wk, "wv": Wv, "wo": Wo, "bo": bo_v,
            "masks": make_masks(c),
        })
    res = run_bass_kernel_spmd(
        nc_prog, in_maps, core_ids=list(range(NC)), trace=trace)
    full = np.empty((T, D), dtype=np.float32)
    for c in range(NC):
        full[q_rows(c)] = res.results[c]["out"]
    return full.reshape(1, T, D), res


def kernel(**inputs) -> np.ndarray:
    out, _ = _run(inputs, trace=False)
    return out



# revision 11
# speedup vs baseline: 1.2811x; 1.2811x over previous
"""Causal MHA (B=1, T=4096, D=768, H=12) on 8 TRN2 NeuronCores.

Strategy (v2)
-------------
- Sequence-parallel over T with row-interleaved q-assignment so every core
  runs the *same* program on identically-shaped causal work:
  core c owns q rows {16*(c+8*t)+u}, i.e. 16-row miniblocks strided by 8.
- No collectives (bass collectives run at ~50-60 GB/s with ~10us/step
  latency floors -- an all-gather of K/V would cost 200us+). Each core
  redundantly projects the FULL K^T and V' from the full x, one 512-key
  chunk at a time, software-pipelined with attention over the previous
  chunk. K^T/V' chunks live only in SBUF (bf16) -- no DRAM roundtrip.
- All matmuls in bf16 (PSUM accumulates f32). Scores ~N(0,1), so softmax
  skips the running max: exp(s/8) never overflows. Unnormalized context +
  denominator (ones-column on V') accumulate in SBUF f32 across chunks.
- S^T per head pair runs as two concurrent K=64 row-tiled matmuls
  (tile_position (0,0)/(64,0) auto-derived from base partitions).
- Per-round causal trim: round r covers q cols [64r, 512), one
  r-independent 192-col mask zeroes the sub-diagonal prefix + diagonal.
- exp batched per (pair, kb) across both heads' PSUM banks, with bank
  packing for the small-N rounds to amortize ACT instruction overhead.
"""
import sys

sys.path.insert(0, "/opt/trn_rl_repo")

import numpy as np

import concourse.bass as bass
import concourse.mybir as mybir
import concourse.tile as tile
from concourse.bass_utils import run_bass_kernel_spmd

P = 128
T, D, H, HD = 4096, 768, 12, 64
NC = 8
SQ = T // NC          # 512 q rows per core
CH = 512              # kv chunk (4 key blocks of 128)
DC = D // P           # 6 contraction chunks
NR = 8                # rounds (chunks)
VROW = H * (HD + 1)   # 780: V' row with ones col per head
BF16 = mybir.dt.bfloat16
F32R = mybir.dt.float32r
F32 = mybir.dt.float32


def q_rows(c):
    t = np.arange(32)
    u = np.arange(16)
    return (16 * (c + 8 * t)[:, None] + u[None, :]).reshape(-1)


def make_mask_ext(c):
    """mask_ext[kap, j, m]: for key block kb = 4r+j, q col (abs) 64r+m:
    valid iff 128j + kap <= 16c + 128*(m//16) + (m%16). r-independent."""
    kap = np.arange(128)[:, None, None]
    j = np.arange(4)[None, :, None]
    m = np.arange(192)[None, None, :]
    valid = (128 * j + kap) <= (16 * c + 128 * (m // 16) + (m % 16))
    return valid.astype(np.float32)


def fix_excess_waits(nc):
    """walrus rejects >1 sync wait per instruction; hoist extras onto NoOps."""
    k = 0
    for f in nc.m.functions:
        for bb in f.blocks:
            insts = bb.instructions
            i = 0
            while i < len(insts):
                ins = insts[i]
                si = getattr(ins, "sync_info", None)
                if si is not None and len(si.on_wait) > 1:
                    for w in si.on_wait[:-1]:
                        nop = mybir.InstNoOp(name=f"W-hoist-{k}", ins=[], outs=[])
                        k += 1
                        nop.engine = ins.engine
                        nop.sync_info = mybir.SyncInfo(on_wait=[w], on_update=[])
                        insts.insert(i, nop)
                        i += 1
                    ins.sync_info = mybir.SyncInfo(
                        on_wait=[si.on_wait[-1]], on_update=list(si.on_update))
                i += 1
    return k


def build(fix_waits=True, debug=False):
    nc = bass.Bass()
    xqt = nc.dram_tensor("xqt", [D, SQ], BF16, kind="ExternalInput")
    xt = nc.dram_tensor("xt", [D, T], BF16, kind="ExternalInput")
    wq = nc.dram_tensor("wq", [D, D], BF16, kind="ExternalInput")
    wk = nc.dram_tensor("wk", [D, D], BF16, kind="ExternalInput")
    wv = nc.dram_tensor("wv", [D, D], BF16, kind="ExternalInput")
    wo = nc.dram_tensor("wo", [D, D], BF16, kind="ExternalInput")
    bo = nc.dram_tensor("bo", [P, D], F32, kind="ExternalInput")
    maskx = nc.dram_tensor("maskx", [P, 4, 192], BF16, kind="ExternalInput")
    out = nc.dram_tensor("out", [SQ, D], F32, kind="ExternalOutput")
    if debug:
        dbg_qt = nc.dram_tensor("dbg_qt", [P, DC, SQ], BF16, kind="ExternalOutput")
        dbg_kt = nc.dram_tensor("dbg_kt", [P, DC, CH], BF16, kind="ExternalOutput")
        dbg_vt = nc.dram_tensor("dbg_vt", [P, 4, VROW], BF16, kind="ExternalOutput")
        dbg_pt = nc.dram_tensor("dbg_pt", [P, 2, 4, SQ], BF16, kind="ExternalOutput")
        dbg_ctxA = nc.dram_tensor("dbg_ctxA", [HD + 1, DC, SQ], F32, kind="ExternalOutput")
        dbg_ctxB = nc.dram_tensor("dbg_ctxB", [HD + 1, DC, SQ], F32, kind="ExternalOutput")
        dbg_ctxt = nc.dram_tensor("dbg_ctxt", [P, DC, SQ], BF16, kind="ExternalOutput")

    EXP = mybir.ActivationFunctionType.Exp

    with tile.TileContext(nc) as tc:
        with (
            tc.tile_pool(name="glob", bufs=1) as glob,
            tc.tile_pool(name="xc", bufs=2) as xcp,
            tc.tile_pool(name="ktp", bufs=2) as ktp,
            tc.tile_pool(name="vtp", bufs=2) as vtp,
            tc.tile_pool(name="att", bufs=2) as att,
            tc.tile_pool(name="ps_s", bufs=2, space="PSUM") as ps_s,
            tc.tile_pool(name="ps_c", bufs=2, space="PSUM") as ps_c,
        ):
            # ---- persistent tiles
            qt = glob.tile([P, DC, SQ], BF16)        # Q^T, head pair i on dc=i
            mask_sb = glob.tile([P, 4, 192], BF16)
            bo_bc = glob.tile([P, D], F32)
            wq_sb = glob.tile([P, DC, D], BF16)
            wk_sb = glob.tile([P, DC, D], BF16)
            wv_sb = glob.tile([P, DC, D], BF16)
            wo_sb = glob.tile([P, DC, D], BF16)
            xq_sb = glob.tile([P, DC, SQ], BF16)
            # unnormalized ctx + denominator row (row 64) per head, f32
            ctxA = glob.tile([HD + 1, DC, SQ], F32)  # even heads 2i
            ctxB = glob.tile([HD + 1, DC, SQ], F32)  # odd heads 2i+1
            ctxt = glob.tile([P, DC, SQ], BF16)      # normalized ctx^T for Wo
            o_sb = glob.tile([P, SQ // P, D], F32)
            ones64 = glob.tile([1, HD], F32R)

            nc.sync.dma_start(wq_sb[:], wq.rearrange("(o p) d -> p o d", p=P))
            nc.sync.dma_start(xq_sb[:], xqt.rearrange("(o p) t -> p o t", p=P))
            nc.sync.dma_start(wk_sb[:], wk.rearrange("(o p) d -> p o d", p=P))
            nc.sync.dma_start(wv_sb[:], wv.rearrange("(o p) d -> p o d", p=P))
            nc.sync.dma_start(wo_sb[:], wo.rearrange("(o p) d -> p o d", p=P))
            nc.sync.dma_start(mask_sb[:], maskx[:])
            nc.sync.dma_start(bo_bc[:], bo[:])
            nc.vector.memset(ones64.bitcast(mybir.dt.uint32), 0x3F800000)

            xtv = xt.rearrange("(o p) t -> p o t", p=P)
            xts = [None] * NR
            kts = [None] * NR
            vts = [None] * NR
            for rr in range(2):
                xts[rr] = xcp.tile([P, DC, CH], BF16, tag="xc", name=f"xt{rr}")
                nc.sync.dma_start(xts[rr][:], xtv[:, :, rr * CH:(rr + 1) * CH])

            def proj_units(r):
                """Closures projecting chunk r's K^T and V' (7 units)."""
                units = []

                def k_unit(q2, r=r):
                    if q2 == 0:
                        kts[r] = ktp.tile([P, DC, CH], BF16, tag="kt", name=f"kt{r}")
                    kt = kts[r]
                    xtc = xts[r]
                    pp = ps_s.tile([P, 2, CH], F32, tag="s")
                    for par in range(2):
                        dc = 2 * q2 + par
                        for ko in range(DC):
                            nc.tensor.matmul(
                                pp[:, par, :], wk_sb[:, ko, dc * P:(dc + 1) * P],
                                xtc[:, ko, :], start=(ko == 0), stop=(ko == DC - 1))
                    nc.vector.tensor_copy(kt[:, 2 * q2:2 * q2 + 2, :], pp[:])

                def v_unit(tc4, r=r):
                    if tc4 == 0:
                        vts[r] = vtp.tile([P, 4, VROW], BF16, tag="vt", name=f"vt{r}")
                        v4i = vts[r].rearrange("p f (h c) -> p f h c", c=HD + 1)
                        nc.vector.memset(v4i[:, :, :, HD:HD + 1], 1.0)
                    vt = vts[r]
                    v4 = vt.rearrange("p f (h c) -> p f h c", c=HD + 1)
                    xtc = xts[r]
                    pp = ps_s.tile([P, 2, CH], F32, tag="s")
                    for nh in range(2):
                        for ko in range(DC):
                            nc.tensor.matmul(
                                pp[:, nh, 0:384], xtc[:, ko, tc4 * P:(tc4 + 1) * P],
                                wv_sb[:, ko, nh * 384:(nh + 1) * 384],
                                start=(ko == 0), stop=(ko == DC - 1))
                    nc.vector.tensor_copy(
                        v4[:, tc4, :, 0:HD].rearrange(
                            "p (n h) c -> p n h c", n=2),
                        pp[:, :, 0:384].rearrange("p n (h c) -> p n h c", c=HD))

                for q2 in range(DC // 2):
                    units.append(lambda q2=q2: k_unit(q2))
                for tc4 in range(4):
                    units.append(lambda tc4=tc4: v_unit(tc4))
                return units

            # ---- Q projection: qt[0:64, i] = head 2i, qt[64:128, i] = 2i+1
            for q2 in range(DC // 2):
                pp = ps_s.tile([P, 2, SQ], F32, tag="s")
                for par in range(2):
                    dc = 2 * q2 + par
                    for ko in range(DC):
                        nc.tensor.matmul(
                            pp[:, par, :], wq_sb[:, ko, dc * P:(dc + 1) * P],
                            xq_sb[:, ko, :], start=(ko == 0), stop=(ko == DC - 1))
                nc.vector.tensor_copy(qt[:, 2 * q2:2 * q2 + 2, :], pp[:])

            # chunk 0 projection up front
            for u in proj_units(0):
                u()

            # ---- rounds: attend over chunk r; interleave projection of r+1
            for r in range(NR):
                scope = nc.named_scope(f"round{r}")
                scope.__enter__()
                N = SQ - 64 * r      # live q cols this round
                q0 = 64 * r
                if r + 2 < NR:
                    xts[r + 2] = xcp.tile([P, DC, CH], BF16, tag="xc",
                                        name=f"xt{r + 2}")
                    nc.sync.dma_start(
                        xts[r + 2][:], xtv[:, :, (r + 2) * CH:(r + 3) * CH])
                kt, vt = kts[r], vts[r]
                if debug and r == 0:
                    nc.sync.dma_start(dbg_kt[:], kt[:])
                    nc.sync.dma_start(dbg_vt[:], vt[:])
                nxt = proj_units(r + 1) if r + 1 < NR else []

                if N > 256:
                    packs = [[0], [1], [2], [3]]
                elif N > 128:
                    packs = [[0, 1], [2, 3]]
                else:
                    packs = [[0, 1, 2, 3]]
                M = min(192, N)

                for i in range(DC):
                    pt = att.tile([P, 2, 4, SQ], BF16, tag="pt")
                    cps = ps_c.tile([P, 2, SQ], F32, tag="c")
                    for pk in packs:
                        sps = ps_s.tile([P, 2, CH], F32, tag="s")
                        for idx, j in enumerate(pk):
                            for hh in range(2):
                                nc.tensor.matmul(
                                    sps[0:P, hh, idx * N:(idx + 1) * N],
                                    kt[64 * hh:64 * hh + 64, i, j * P:(j + 1) * P],
                                    qt[64 * hh:64 * hh + 64, i, q0:SQ],
                                    start=True, stop=True)
                        W = len(pk) * N
                        j0 = pk[0]
                        nc.scalar.activation(
                            pt[:, :, j0:j0 + len(pk), 0:N], sps[:, :, 0:W],
                            EXP, scale=0.125)
                        # causal mask for this pack's key blocks
                        for hh in range(2):
                            nc.vector.tensor_mul(
                                pt[:, hh, j0:j0 + len(pk), 0:M],
                                pt[:, hh, j0:j0 + len(pk), 0:M],
                                mask_sb[:, j0:j0 + len(pk), 0:M])
                        # PV for this pack
                        for j in pk:
                            for hh in range(2):
                                h = 2 * i + hh
                                nc.tensor.matmul(
                                    cps[0:HD + 1, hh, 0:N],
                                    vt[:, j, h * (HD + 1):(h + 1) * (HD + 1)],
                                    pt[:, hh, j, 0:N],
                                    start=(j == 0), stop=(j == 3),
                                    skip_group_check=True)
                    if debug and r == 0 and i == 0:
                        nc.sync.dma_start(dbg_pt[:], pt[:])
                    # drain pair ctx into SBUF accumulators
                    for hh, ctx in ((0, ctxA), (1, ctxB)):
                        if r == 0:
                            nc.vector.tensor_copy(
                                ctx[:, i, :], cps[0:HD + 1, hh, :])
                        else:
                            nc.vector.tensor_add(
                                ctx[:, i, q0:SQ], ctx[:, i, q0:SQ],
                                cps[0:HD + 1, hh, 0:N])
                    # interleave next-chunk projection work to fill PE stalls
                    if i < 5:
                        if i < len(nxt):
                            nxt[i]()
                    else:
                        for u in nxt[5:]:
                            u()
                scope.__exit__(None, None, None)

            if debug:
                nc.sync.dma_start(dbg_qt[:], qt[:])
                nc.sync.dma_start(dbg_ctxA[:], ctxA[:])
                nc.sync.dma_start(dbg_ctxB[:], ctxB[:])

            # ---- normalize: ctxt = ctx~ / den  (den broadcast via PE)
            with tc.tile_pool(name="nrm", bufs=2) as nrm:
                for i in range(DC):
                    for hh, ctx in ((0, ctxA), (1, ctxB)):
                        rec = nrm.tile([1, SQ], F32R, tag="rec")
                        with nc.allow_low_precision(reason="f32r broadcast"):
                            nc.vector.reciprocal(rec[:], ctx[HD:HD + 1, i, :])
                        bc = ps_c.tile([HD, 2, SQ], F32, tag="c")
                        nc.tensor.matmul(
                            bc[:, 0, :], ones64[:], rec[:],
                            start=True, stop=True)
                        nc.vector.tensor_mul(
                            ctxt[64 * hh:64 * hh + 64, i, :],
                            ctx[0:HD, i, :], bc[:, 0, :])

            # ---- output projection
            for tc4 in range(SQ // P):
                op = ps_s.tile([P, 2, CH], F32, tag="s")
                for nh in range(2):
                    for dc in range(DC):
                        nc.tensor.matmul(
                            op[:, nh, 0:384], ctxt[:, dc, tc4 * P:(tc4 + 1) * P],
                            wo_sb[:, dc, nh * 384:(nh + 1) * 384],
                            start=(dc == 0), stop=(dc == DC - 1))
                nc.vector.tensor_add(
                    o_sb[:, tc4, :].rearrange("p (n c) -> p n c", n=2),
                    op[:, :, 0:384],
                    bo_bc.rearrange("p (n c) -> p n c", n=2))
            if debug:
                nc.sync.dma_start(dbg_ctxt[:], ctxt[:])
            nc.sync.dma_start(out.rearrange("(o p) d -> p o d", p=P), o_sb[:])

    if fix_waits:
        fix_excess_waits(nc)
    return nc


_NC_CACHE = None


def _get_nc():
    global _NC_CACHE
    if _NC_CACHE is None:
        _NC_CACHE = build()
    return _NC_CACHE


def _run(inputs, trace=False):
    import ml_dtypes
    bf16 = ml_dtypes.bfloat16

    x = np.asarray(inputs["x"], dtype=np.float32)
    Wq = np.asarray(inputs["Wq"], dtype=np.float32).astype(bf16)
    Wk = np.asarray(inputs["Wk"], dtype=np.float32).astype(bf16)
    Wv = np.asarray(inputs["Wv"], dtype=np.float32).astype(bf16)
    Wo = np.asarray(inputs["Wo"], dtype=np.float32).astype(bf16)
    bo_v = np.ascontiguousarray(
        np.broadcast_to(np.asarray(inputs["bo"], dtype=np.float32).reshape(1, D),
                        (P, D)))
    xf = x.reshape(T, D)
    xt_full = np.ascontiguousarray(xf.T).astype(bf16)

    nc_prog = _get_nc()
    in_maps = []
    for c in range(NC):
        rows = q_rows(c)
        in_maps.append({
            "xqt": np.ascontiguousarray(xf[rows].T).astype(bf16),
            "xt": xt_full,
            "wq": Wq, "wk": Wk, "wv": Wv, "wo": Wo, "bo": bo_v,
            "maskx": make_mask_ext(c).astype(bf16),
        })
    res = run_bass_kernel_spmd(
        nc_prog, in_maps, core_ids=list(range(NC)), trace=trace)
    full = np.empty((T, D), dtype=np.float32)
    for c in range(NC):
        full[q_rows(c)] = res.results[c]["out"]
    return full.reshape(1, T, D), res


def kernel(**inputs) -> np.ndarray:
    out, _ = _run(inputs, trace=False)
    return out
